# revision 10
# baseline (speedup 1.0000x reference)
"""Trainium2 Bass kernel for nn_Actor (gnn_message_passing tree aggregation).

Strategy:
  - Data-parallel over batch B=4096 across 8 NeuronCores (512 rows each);
    weights replicated. No collectives - pure SPMD.
  - Activations live in SBUF "transposed" (feature on partitions, batch on
    free dim). Tree concat of 4 sibling embeddings along the feature dim is
    then free: the 4 child tiles are directly the 4 K-chunks of the parent
    aggregator's first matmul.
  - Matmuls run in bf16 (fp32 PSUM accumulate). Weights are pre-cast and
    pre-packed on the host into the exact SBUF layout (one contiguous
    [128, X] bundle per tree group), halving weight DMA traffic and making
    every weight DMA a single large contiguous transfer. Biases are
    pre-transposed on the host into one [128, 384] f32 bundle.
  - Weight bundles are streamed from HBM group-by-group, triple buffered.
  - Output embeds [B,85,128] are batch-major, so each node embedding is
    PE-transposed (via identity matmul) back to batch-major before DMA out.
  - Bias+ReLU epilogues alternate between ScalarE (activation) and VectorE
    (fused tensor_scalar add+max) to balance the two engines.
"""

import numpy as np
import ml_dtypes

import concourse.bass as bass
import concourse.tile as tile
from concourse import bacc, mybir
from concourse.bass import ts
from concourse.bass_utils import run_bass_kernel_spmd
from concourse.masks import make_identity

AF = mybir.ActivationFunctionType
ALU = mybir.AluOpType
F32 = mybir.dt.float32
BF16 = mybir.dt.bfloat16
NP_BF16 = np.dtype(ml_dtypes.bfloat16)

# Problem shapes (hardcoded; must match the reference)
B, S, D, E, F, A = 4096, 256, 128, 64, 4, 32
CD = D * F                      # 512
NODES = 85                      # 64 enc + 16 + 4 + 1
N_CORES = 8
BL = B // N_CORES               # 512 rows per core
NBC = BL // 128                 # 4 batch chunks of 128

# bias bundle column offsets (host packing must match kernel reads)
EB1, EBH, EBO = 0, 64, 128
A0B1, A0BH, A0BO = 192, 256, 320
A1B1, A1BH, A1BO = 336, 352, 368
A2B1, A2BH, A2BO = 372, 376, 380
HB1, HBH, HBO = 381, 382, 383
NBIAS = 384

DRAM_SPECS = {
    "state_t": ((2, 128, BL), BF16),
    "enc_pack": ((16, 128, 2048), BF16),
    "agg_pack": ((21, 128, 4608), BF16),
    "head_pack": ((128, 288), BF16),
    "bias_pack": ((128, NBIAS), F32),
}


def build_kernel():
    nc = bacc.Bacc("TRN2", target_bir_lowering=False, debug=False)

    dram = {}
    for name, (shape, dt) in DRAM_SPECS.items():
        dram[name] = nc.dram_tensor(name, list(shape), dt, kind="ExternalInput").ap()
    embeds = nc.dram_tensor("embeds", [BL, NODES, D], F32, kind="ExternalOutput").ap()
    action = nc.dram_tensor("action", [BL, A], F32, kind="ExternalOutput").ap()

    # batch-chunked views of the outputs: [128, chunk, node, d]
    embeds_v = embeds.rearrange("(c p) n d -> p c n d", p=128)
    action_v = action.rearrange("(c p) a -> p c a", p=128)

    with tile.TileContext(nc) as tc:
        with (
            tc.tile_pool(name="const", bufs=1) as const,
            tc.tile_pool(name="wenc", bufs=3) as wenc,
            tc.tile_pool(name="wagg", bufs=3) as wagg,
            tc.tile_pool(name="encout", bufs=10) as encout,
            tc.tile_pool(name="hid", bufs=6) as hid,
            tc.tile_pool(name="xpool", bufs=10) as xpool,
            tc.tile_pool(name="a0out", bufs=16) as a0out,
            tc.tile_pool(name="a1out", bufs=5) as a1out,
            tc.tile_pool(name="smallp", bufs=1) as smallp,
            tc.tile_pool(name="natp", bufs=3) as natp,
            tc.tile_pool(name="anatp", bufs=2) as anatp,
            tc.tile_pool(name="psmm", bufs=8, space="PSUM") as psmm,
        ):
            ident = const.tile([128, 128], BF16)
            make_identity(nc, ident)

            stT = const.tile([128, 2, BL], BF16, tag="stT")
            nc.scalar.dma_start(stT, dram["state_t"].rearrange("s p b -> p s b"))
            bias = const.tile([128, NBIAS], F32, tag="bias")
            nc.scalar.dma_start(bias, dram["bias_pack"])

            # --- helpers -------------------------------------------------
            act_counter = [0]

            def act_bias(out, ps, bias_ap, relu):
                """bias-add (+ReLU) epilogue, alternating ScalarE / VectorE."""
                i = act_counter[0]
                act_counter[0] += 1
                if i % 5 < 2:
                    if relu:
                        nc.vector.tensor_scalar(
                            out, ps, bias_ap, 0.0, ALU.add, ALU.max
                        )
                    else:
                        nc.vector.tensor_scalar_add(out, ps, bias_ap)
                else:
                    nc.scalar.activation(
                        out, ps, AF.Relu if relu else AF.Identity, bias=bias_ap
                    )

            def mm_acc(lhs_chunks, rhs_list, out_par=128):
                ps = psmm.tile([128, BL], F32, tag="ps")
                nk = len(rhs_list)
                for k in range(nk):
                    nc.tensor.matmul(
                        ps[0:out_par, :], lhs_chunks[k], rhs_list[k],
                        start=(k == 0), stop=(k == nk - 1),
                    )
                return ps

            def transpose_to_nat(src_T, dst_nat, valid=128):
                """xbar DMA-transpose [feat, b] -> batch-major [128, NBC, feat].
                Runs on the Sync HWDGE ring (kept free of normal DMAs so the
                xbar mode never has to toggle mid-stream)."""
                for c in range(NBC):
                    nc.sync.dma_start(
                        dst_nat[:, c, 0:valid], src_T[0:valid, ts(c, 128)],
                        transpose=True,
                    )

            def fnn_3layer(x_list, aggw, boff, gi, nloc, out_pool, out_tag):
                """Aggregator FNN CD->CD->CD->D on transposed acts.
                aggw: [128, 4608] bundle; boff: (b1,bh,bo) col offsets;
                gi: node index in level; nloc: nodes in level."""
                w1 = aggw[:, 0:2048].rearrange("p (k m) -> p k m", k=4)
                wh = aggw[:, 2048:4096].rearrange("p (k m) -> p k m", k=4)
                wo = aggw[:, 4096:4608].rearrange("p (k m) -> p k m", k=4)
                ob1, obh, obo = boff
                x1 = []
                for m in range(4):
                    ps = mm_acc([w1[:, k, ts(m, 128)] for k in range(4)], x_list)
                    t = xpool.tile([128, BL], BF16, tag="x1")
                    c = ob1 + m * nloc + gi
                    act_bias(t, ps, bias[:, c:c + 1], relu=True)
                    x1.append(t)
                x2 = []
                for m in range(4):
                    ps = mm_acc([wh[:, k, ts(m, 128)] for k in range(4)], x1)
                    t = xpool.tile([128, BL], BF16, tag="x2")
                    c = obh + m * nloc + gi
                    act_bias(t, ps, bias[:, c:c + 1], relu=True)
                    x2.append(t)
                ps = mm_acc([wo[:, k, :] for k in range(4)], x2)
                out = out_pool.tile([128, BL], BF16, tag=out_tag)
                act_bias(out, ps, bias[:, obo + gi:obo + gi + 1], relu=False)
                return out

            # ================= encoders + level-0 aggregators =============
            a0_tiles = []
            a0nat = None
            for g in range(16):
                encw = wenc.tile([128, 2048], BF16, tag="encw")
                nc.scalar.dma_start(encw, dram["enc_pack"][g])
                ew1 = encw[:, 0:1024].rearrange("p (e c f) -> p e c f", e=4, c=2)
                ewh = encw[:, 1024:1536].rearrange("p (e f) -> p e f", e=4)
                ewo = encw[:, 1536:2048].rearrange("p (e f) -> p e f", e=4)

                # breadth-first over the 4 encoders: keeps 4 independent
                # matmul chains in the scheduler window so PE never stalls
                # on a single epilogue.
                h1s, h2s, e_tiles = [], [], []
                for e in range(4):
                    n = 4 * g + e
                    ps = mm_acc(
                        [ew1[:, e, c, :] for c in range(2)],
                        [stT[:, c, :] for c in range(2)],
                    )
                    h1 = hid.tile([128, BL], BF16, tag="h1")
                    act_bias(h1, ps, bias[:, EB1 + n:EB1 + n + 1], relu=True)
                    h1s.append(h1)
                for e in range(4):
                    n = 4 * g + e
                    ps = mm_acc([ewh[:, e, :]], [h1s[e]])
                    h2 = hid.tile([128, BL], BF16, tag="h2")
                    act_bias(h2, ps, bias[:, EBH + n:EBH + n + 1], relu=True)
                    h2s.append(h2)
                for e in range(4):
                    n = 4 * g + e
                    ps = mm_acc([ewo[:, e, :]], [h2s[e]])
                    eT = encout.tile([128, BL], BF16, tag="eT")
                    act_bias(eT, ps, bias[:, EBO + n:EBO + n + 1], relu=False)
                    e_tiles.append(eT)

                # -- level-0 aggregator for this group (critical path) --
                aggw = wagg.tile([128, 4608], BF16, tag="aggw")
                nc.scalar.dma_start(aggw, dram["agg_pack"][g])
                aT = fnn_3layer(
                    e_tiles, aggw, (A0B1, A0BH, A0BO), g, 16, a0out, "a0"
                )

                # encoder embeds output (off the critical path: emitted after
                # the aggregator so its matmuls get scheduler priority)
                enat = natp.tile([128, NBC, 4, 128], BF16, tag="enat")
                for e in range(4):
                    transpose_to_nat(e_tiles[e], enat[:, :, e, :])
                nc.gpsimd.dma_start(embeds_v[:, :, ts(g, 4), :], enat)
                a0_tiles.append(aT)
                if g % 4 == 0:
                    a0nat = anatp.tile([128, NBC, 4, 128], BF16, tag="anat")
                transpose_to_nat(aT, a0nat[:, :, g % 4, :])
                if g % 4 == 3:
                    nc.gpsimd.dma_start(
                        embeds_v[:, :, ts(16 + g // 4, 4), :], a0nat
                    )

            # ================= level-1 aggregators ========================
            a1_tiles = []
            a1nat = anatp.tile([128, NBC, 4, 128], BF16, tag="anat")
            for j in range(4):
                aggw = wagg.tile([128, 4608], BF16, tag="aggw")
                nc.scalar.dma_start(aggw, dram["agg_pack"][16 + j])
                aT = fnn_3layer(
                    a0_tiles[4 * j:4 * j + 4], aggw, (A1B1, A1BH, A1BO),
                    j, 4, a1out, "a1",
                )
                a1_tiles.append(aT)
                transpose_to_nat(aT, a1nat[:, :, j, :])
            nc.gpsimd.dma_start(embeds_v[:, :, ts(20, 4), :], a1nat)

            # ================= level-2 (root) =============================
            aggw = wagg.tile([128, 4608], BF16, tag="aggw")
            nc.scalar.dma_start(aggw, dram["agg_pack"][20])
            rootT = fnn_3layer(
                a1_tiles, aggw, (A2B1, A2BH, A2BO), 0, 1, smallp, "root"
            )
            rnat = smallp.tile([128, NBC, 1, 128], BF16, tag="rnat")
            transpose_to_nat(rootT, rnat[:, :, 0, :])
            nc.gpsimd.dma_start(embeds_v[:, :, 84:85, :], rnat)

            # ================= head =======================================
            headw = smallp.tile([128, 288], BF16, tag="headw")
            nc.scalar.dma_start(headw, dram["head_pack"])
            hw1, hwh, hwo = headw[:, 0:128], headw[:, 128:256], headw[:, 256:288]

            ps = mm_acc([hw1], [rootT])
            hh1 = hid.tile([128, BL], BF16, tag="h1")
            act_bias(hh1, ps, bias[:, HB1:HB1 + 1], relu=True)
            ps = mm_acc([hwh], [hh1])
            hh2 = hid.tile([128, BL], BF16, tag="h2")
            act_bias(hh2, ps, bias[:, HBH:HBH + 1], relu=True)
            ps = mm_acc([hwo], [hh2], out_par=A)
            actT = smallp.tile([A, BL], BF16, tag="actT")
            nc.scalar.activation(
                actT, ps[0:A, :], AF.Tanh, bias=bias[0:A, HBO:HBO + 1]
            )

            anat = smallp.tile([128, NBC, A], BF16, tag="act_nat")
            for c in range(NBC):
                nc.sync.dma_start(
                    anat[:, c, :], actT[:, ts(c, 128)], transpose=True
                )
            nc.gpsimd.dma_start(action_v, anat)

    nc.compile()
    return nc


def pack_inputs(full):
    """Host-side: cast+pack weights into SBUF-layout bundles (bf16) and
    biases into one transposed f32 bundle."""
    f32 = np.float32
    enc_pack = np.empty((16, 128, 2048), NP_BF16)
    for g in range(16):
        sl = slice(4 * g, 4 * g + 4)
        ew1 = (full["enc_w1"][sl].reshape(4, 2, 128, 128)
               .transpose(2, 0, 1, 3).reshape(128, 1024))
        ewh = full["enc_wh"][sl].transpose(1, 0, 2).reshape(128, 512)
        ewo = full["enc_wo"][sl].transpose(1, 0, 2).reshape(128, 512)
        enc_pack[g] = np.concatenate([ew1, ewh, ewo], axis=1).astype(NP_BF16)

    agg_pack = np.empty((21, 128, 4608), NP_BF16)
    idx = 0
    for lvl in ("agg0", "agg1", "agg2"):
        for gi in range(full[f"{lvl}_w1"].shape[0]):
            w1 = (full[f"{lvl}_w1"][gi].reshape(4, 128, 512)
                  .transpose(1, 0, 2).reshape(128, 2048))
            wh = (full[f"{lvl}_wh"][gi].reshape(4, 128, 512)
                  .transpose(1, 0, 2).reshape(128, 2048))
            wo = (full[f"{lvl}_wo"][gi].reshape(4, 128, 128)
                  .transpose(1, 0, 2).reshape(128, 512))
            agg_pack[idx] = np.concatenate([w1, wh, wo], axis=1).astype(NP_BF16)
            idx += 1

    head_pack = np.concatenate(
        [full["head_w1"], full["head_wh"], full["head_wo"]], axis=1
    ).astype(NP_BF16)

    def t2(b):     # [n, 128] -> [128, n]
        return np.ascontiguousarray(b.T)

    def t4(b):     # [n, 512] -> [128, 4n] laid out as (c, i)
        n = b.shape[0]
        return b.reshape(n, 4, 128).transpose(2, 1, 0).reshape(128, 4 * n)

    hbo = np.zeros((128, 1), f32)
    hbo[0:A, 0] = full["head_bo"]
    bias_pack = np.concatenate([
        t2(full["enc_b1"]), t2(full["enc_bh"]), t2(full["enc_bo"]),
        t4(full["agg0_b1"]), t4(full["agg0_bh"]), t2(full["agg0_bo"]),
        t4(full["agg1_b1"]), t4(full["agg1_bh"]), t2(full["agg1_bo"]),
        t4(full["agg2_b1"]), t4(full["agg2_bh"]), t2(full["agg2_bo"]),
        full["head_b1"][:, None], full["head_bh"][:, None], hbo,
    ], axis=1).astype(f32)
    assert bias_pack.shape == (128, NBIAS)

    return {
        "enc_pack": enc_pack,
        "agg_pack": agg_pack,
        "head_pack": np.ascontiguousarray(head_pack),
        "bias_pack": np.ascontiguousarray(bias_pack),
    }


_NC_CACHE = None


def _get_nc():
    global _NC_CACHE
    if _NC_CACHE is None:
        _NC_CACHE = build_kernel()
    return _NC_CACHE


def run_sharded(inputs, trace=False, tmpdir=None):
    """inputs: dict of full-size np arrays. Returns (embeds, action, results)."""
    nc = _get_nc()
    full = {
        k: np.ascontiguousarray(np.asarray(v, dtype=np.float32))
        for k, v in inputs.items()
    }
    packed = pack_inputs(full)
    in_maps = []
    for i in range(N_CORES):
        st = full["state"][i * BL:(i + 1) * BL]            # [BL, S]
        state_t = np.ascontiguousarray(st.T).reshape(2, 128, BL).astype(NP_BF16)
        m = dict(packed)
        m["state_t"] = state_t
        in_maps.append(m)
    res = run_bass_kernel_spmd(
        nc, in_maps, core_ids=list(range(N_CORES)), trace=trace, tmpdir=tmpdir
    )
    embeds = np.concatenate([res.results[i]["embeds"] for i in range(N_CORES)], axis=0)
    action = np.concatenate([res.results[i]["action"] for i in range(N_CORES)], axis=0)
    return embeds, action, res


def kernel(**inputs):
    embeds, action, _ = run_sharded(inputs)
    return embeds, action


# revision 11
# speedup vs baseline: 2.1272x; 2.1272x over previous
"""Trainium2 Bass kernel for nn_Actor (gnn_message_passing tree aggregation).

Strategy:
  - Data-parallel over batch B=4096 across 8 NeuronCores (512 rows each);
    weights replicated. No collectives - pure SPMD.
  - Activations live in SBUF "transposed" (feature on partitions, batch on
    free dim). Tree concat of 4 sibling embeddings along the feature dim is
    then free: the 4 child tiles are directly the 4 K-chunks of the parent
    aggregator's first matmul.
  - Matmuls run in bf16 (fp32 PSUM accumulate). Weights are pre-cast and
    pre-packed on the host into the exact SBUF layout (one contiguous
    [128, X] bundle per tree group), halving weight DMA traffic and making
    every weight DMA a single large contiguous transfer. Biases are
    pre-transposed on the host into one [128, 384] f32 bundle.
  - Weight bundles are streamed from HBM group-by-group, triple buffered.
  - Output embeds [B,85,128] are batch-major, so each node embedding is
    PE-transposed (via identity matmul) back to batch-major before DMA out.
  - Bias+ReLU epilogues alternate between ScalarE (activation) and VectorE
    (fused tensor_scalar add+max) to balance the two engines.
"""

import numpy as np
import ml_dtypes

import concourse.bass as bass
import concourse.tile as tile
from concourse import bacc, mybir
from concourse.bass import ts
from concourse.bass_utils import run_bass_kernel_spmd
from concourse.masks import make_identity

AF = mybir.ActivationFunctionType
ALU = mybir.AluOpType
F32 = mybir.dt.float32
BF16 = mybir.dt.bfloat16
NP_BF16 = np.dtype(ml_dtypes.bfloat16)

# Problem shapes (hardcoded; must match the reference)
B, S, D, E, F, A = 4096, 256, 128, 64, 4, 32
CD = D * F                      # 512
NODES = 85                      # 64 enc + 16 + 4 + 1
N_CORES = 8
BL = B // N_CORES               # 512 rows per core
NBC = BL // 128                 # 4 batch chunks of 128

# bias bundle column offsets (host packing must match kernel reads)
EB1, EBH, EBO = 0, 64, 128
A0B1, A0BH, A0BO = 192, 256, 320
A1B1, A1BH, A1BO = 336, 352, 368
A2B1, A2BH, A2BO = 372, 376, 380
HB1, HBH, HBO = 381, 382, 383
NBIAS = 384

DRAM_SPECS = {
    "state_t": ((2, 128, BL), BF16),
    "enc_pack": ((16, 128, 2048), BF16),
    "agg_pack": ((21, 128, 4608), BF16),
    "head_pack": ((128, 288), BF16),
    "bias_pack": ((128, NBIAS), F32),
}


def build_kernel():
    nc = bacc.Bacc("TRN2", target_bir_lowering=False, debug=False)

    dram = {}
    for name, (shape, dt) in DRAM_SPECS.items():
        dram[name] = nc.dram_tensor(name, list(shape), dt, kind="ExternalInput").ap()
    embeds = nc.dram_tensor("embeds", [BL, NODES, D], F32, kind="ExternalOutput").ap()
    action = nc.dram_tensor("action", [BL, A], F32, kind="ExternalOutput").ap()

    # batch-chunked views of the outputs: [128, chunk, node, d]
    embeds_v = embeds.rearrange("(c p) n d -> p c n d", p=128)
    action_v = action.rearrange("(c p) a -> p c a", p=128)

    with tile.TileContext(nc) as tc:
        with (
            tc.tile_pool(name="const", bufs=1) as const,
            tc.tile_pool(name="wenc", bufs=4) as wenc,
            tc.tile_pool(name="wagg", bufs=4) as wagg,
            tc.tile_pool(name="encout", bufs=12) as encout,
            tc.tile_pool(name="hid", bufs=8) as hid,
            tc.tile_pool(name="xpool", bufs=12) as xpool,
            tc.tile_pool(name="a0out", bufs=16) as a0out,
            tc.tile_pool(name="a1out", bufs=5) as a1out,
            tc.tile_pool(name="smallp", bufs=1) as smallp,
            tc.tile_pool(name="natp", bufs=4) as natp,
            tc.tile_pool(name="anatp", bufs=3) as anatp,
            tc.tile_pool(name="psmm", bufs=6, space="PSUM") as psmm,
            tc.tile_pool(name="pstr", bufs=2, space="PSUM") as pstr,
        ):
            ident = const.tile([128, 128], BF16)
            make_identity(nc, ident)

            stT = const.tile([128, 2, BL], BF16, tag="stT")
            nc.sync.dma_start(stT, dram["state_t"].rearrange("s p b -> p s b"))
            bias = const.tile([128, NBIAS], F32, tag="bias")
            nc.sync.dma_start(bias, dram["bias_pack"])

            # --- helpers -------------------------------------------------
            act_counter = [0]

            def act_bias(out, ps, bias_ap, relu):
                """bias-add (+ReLU) epilogue, alternating ScalarE / VectorE."""
                i = act_counter[0]
                act_counter[0] += 1
                if i % 5 < 2:
                    if relu:
                        nc.vector.tensor_scalar(
                            out, ps, bias_ap, 0.0, ALU.add, ALU.max
                        )
                    else:
                        nc.vector.tensor_scalar_add(out, ps, bias_ap)
                else:
                    nc.scalar.activation(
                        out, ps, AF.Relu if relu else AF.Identity, bias=bias_ap
                    )

            def mm_acc(lhs_chunks, rhs_list, out_par=128):
                ps = psmm.tile([128, BL], F32, tag="ps")
                nk = len(rhs_list)
                for k in range(nk):
                    nc.tensor.matmul(
                        ps[0:out_par, :], lhs_chunks[k], rhs_list[k],
                        start=(k == 0), stop=(k == nk - 1),
                    )
                return ps

            def transpose_to_nat(src_T, dst_nat, valid=128):
                """PE-transpose [feat, b] -> batch-major [128, NBC, feat]."""
                tp = pstr.tile([128, NBC, 128], BF16, tag="tps")
                for c in range(NBC):
                    nc.tensor.transpose(
                        tp[:, c, 0:valid], src_T[0:valid, ts(c, 128)],
                        ident[0:valid, 0:valid],
                    )
                nc.vector.tensor_copy(dst_nat, tp[:, :, 0:valid])

            def fnn_3layer(x_list, aggw, boff, gi, nloc, out_pool, out_tag):
                """Aggregator FNN CD->CD->CD->D on transposed acts.
                aggw: [128, 4608] bundle; boff: (b1,bh,bo) col offsets;
                gi: node index in level; nloc: nodes in level."""
                w1 = aggw[:, 0:2048].rearrange("p (k m) -> p k m", k=4)
                wh = aggw[:, 2048:4096].rearrange("p (k m) -> p k m", k=4)
                wo = aggw[:, 4096:4608].rearrange("p (k m) -> p k m", k=4)
                ob1, obh, obo = boff
                x1 = []
                for m in range(4):
                    ps = mm_acc([w1[:, k, ts(m, 128)] for k in range(4)], x_list)
                    t = xpool.tile([128, BL], BF16, tag="x1")
                    c = ob1 + m * nloc + gi
                    act_bias(t, ps, bias[:, c:c + 1], relu=True)
                    x1.append(t)
                x2 = []
                for m in range(4):
                    ps = mm_acc([wh[:, k, ts(m, 128)] for k in range(4)], x1)
                    t = xpool.tile([128, BL], BF16, tag="x2")
                    c = obh + m * nloc + gi
                    act_bias(t, ps, bias[:, c:c + 1], relu=True)
                    x2.append(t)
                ps = mm_acc([wo[:, k, :] for k in range(4)], x2)
                out = out_pool.tile([128, BL], BF16, tag=out_tag)
                act_bias(out, ps, bias[:, obo + gi:obo + gi + 1], relu=False)
                return out

            # ================= encoders + level-0 aggregators =============
            a0_tiles = []
            a0nat = None
            for g in range(16):
                encw = wenc.tile([128, 2048], BF16, tag="encw")
                nc.sync.dma_start(encw, dram["enc_pack"][g])
                ew1 = encw[:, 0:1024].rearrange("p (e c f) -> p e c f", e=4, c=2)
                ewh = encw[:, 1024:1536].rearrange("p (e f) -> p e f", e=4)
                ewo = encw[:, 1536:2048].rearrange("p (e f) -> p e f", e=4)

                # breadth-first over the 4 encoders: keeps 4 independent
                # matmul chains in the scheduler window so PE never stalls
                # on a single epilogue.
                h1s, h2s, e_tiles = [], [], []
                for e in range(4):
                    n = 4 * g + e
                    ps = mm_acc(
                        [ew1[:, e, c, :] for c in range(2)],
                        [stT[:, c, :] for c in range(2)],
                    )
                    h1 = hid.tile([128, BL], BF16, tag="h1")
                    act_bias(h1, ps, bias[:, EB1 + n:EB1 + n + 1], relu=True)
                    h1s.append(h1)
                for e in range(4):
                    n = 4 * g + e
                    ps = mm_acc([ewh[:, e, :]], [h1s[e]])
                    h2 = hid.tile([128, BL], BF16, tag="h2")
                    act_bias(h2, ps, bias[:, EBH + n:EBH + n + 1], relu=True)
                    h2s.append(h2)
                for e in range(4):
                    n = 4 * g + e
                    ps = mm_acc([ewo[:, e, :]], [h2s[e]])
                    eT = encout.tile([128, BL], BF16, tag="eT")
                    act_bias(eT, ps, bias[:, EBO + n:EBO + n + 1], relu=False)
                    e_tiles.append(eT)

                # -- level-0 aggregator for this group (critical path) --
                aggw = wagg.tile([128, 4608], BF16, tag="aggw")
                nc.sync.dma_start(aggw, dram["agg_pack"][g])
                aT = fnn_3layer(
                    e_tiles, aggw, (A0B1, A0BH, A0BO), g, 16, a0out, "a0"
                )

                # encoder embeds output (off the critical path: emitted after
                # the aggregator so its matmuls get scheduler priority)
                enat = natp.tile([128, NBC, 4, 128], F32, tag="enat")
                for e in range(4):
                    transpose_to_nat(e_tiles[e], enat[:, :, e, :])
                nc.scalar.dma_start(embeds_v[:, :, ts(g, 4), :], enat)
                a0_tiles.append(aT)
                if g % 4 == 0:
                    a0nat = anatp.tile([128, NBC, 4, 128], F32, tag="anat")
                transpose_to_nat(aT, a0nat[:, :, g % 4, :])
                if g % 4 == 3:
                    nc.scalar.dma_start(
                        embeds_v[:, :, ts(16 + g // 4, 4), :], a0nat
                    )

            # ================= level-1 aggregators ========================
            a1_tiles = []
            a1nat = anatp.tile([128, NBC, 4, 128], F32, tag="anat")
            for j in range(4):
                aggw = wagg.tile([128, 4608], BF16, tag="aggw")
                nc.sync.dma_start(aggw, dram["agg_pack"][16 + j])
                aT = fnn_3layer(
                    a0_tiles[4 * j:4 * j + 4], aggw, (A1B1, A1BH, A1BO),
                    j, 4, a1out, "a1",
                )
                a1_tiles.append(aT)
                transpose_to_nat(aT, a1nat[:, :, j, :])
            nc.scalar.dma_start(embeds_v[:, :, ts(20, 4), :], a1nat)

            # ================= level-2 (root) =============================
            aggw = wagg.tile([128, 4608], BF16, tag="aggw")
            nc.sync.dma_start(aggw, dram["agg_pack"][20])
            rootT = fnn_3layer(
                a1_tiles, aggw, (A2B1, A2BH, A2BO), 0, 1, smallp, "root"
            )
            rnat = smallp.tile([128, NBC, 1, 128], F32, tag="rnat")
            transpose_to_nat(rootT, rnat[:, :, 0, :])
            nc.scalar.dma_start(embeds_v[:, :, 84:85, :], rnat)

            # ================= head =======================================
            headw = smallp.tile([128, 288], BF16, tag="headw")
            nc.sync.dma_start(headw, dram["head_pack"])
            hw1, hwh, hwo = headw[:, 0:128], headw[:, 128:256], headw[:, 256:288]

            ps = mm_acc([hw1], [rootT])
            hh1 = hid.tile([128, BL], BF16, tag="h1")
            act_bias(hh1, ps, bias[:, HB1:HB1 + 1], relu=True)
            ps = mm_acc([hwh], [hh1])
            hh2 = hid.tile([128, BL], BF16, tag="h2")
            act_bias(hh2, ps, bias[:, HBH:HBH + 1], relu=True)
            ps = mm_acc([hwo], [hh2], out_par=A)
            actT = smallp.tile([A, BL], BF16, tag="actT")
            nc.scalar.activation(
                actT, ps[0:A, :], AF.Tanh, bias=bias[0:A, HBO:HBO + 1]
            )

            anat = smallp.tile([128, NBC, A], F32, tag="act_nat")
            tp = pstr.tile([128, NBC, 128], BF16, tag="tps")
            for c in range(NBC):
                nc.tensor.transpose(
                    tp[:, c, 0:A], actT[:, ts(c, 128)], ident[0:A, 0:A]
                )
            nc.vector.tensor_copy(anat, tp[:, :, 0:A])
            nc.scalar.dma_start(action_v, anat)

    nc.compile()
    return nc


def pack_inputs(full):
    """Host-side: cast+pack weights into SBUF-layout bundles (bf16) and
    biases into one transposed f32 bundle."""
    f32 = np.float32
    enc_pack = np.empty((16, 128, 2048), NP_BF16)
    for g in range(16):
        sl = slice(4 * g, 4 * g + 4)
        ew1 = (full["enc_w1"][sl].reshape(4, 2, 128, 128)
               .transpose(2, 0, 1, 3).reshape(128, 1024))
        ewh = full["enc_wh"][sl].transpose(1, 0, 2).reshape(128, 512)
        ewo = full["enc_wo"][sl].transpose(1, 0, 2).reshape(128, 512)
        enc_pack[g] = np.concatenate([ew1, ewh, ewo], axis=1).astype(NP_BF16)

    agg_pack = np.empty((21, 128, 4608), NP_BF16)
    idx = 0
    for lvl in ("agg0", "agg1", "agg2"):
        for gi in range(full[f"{lvl}_w1"].shape[0]):
            w1 = (full[f"{lvl}_w1"][gi].reshape(4, 128, 512)
                  .transpose(1, 0, 2).reshape(128, 2048))
            wh = (full[f"{lvl}_wh"][gi].reshape(4, 128, 512)
                  .transpose(1, 0, 2).reshape(128, 2048))
            wo = (full[f"{lvl}_wo"][gi].reshape(4, 128, 128)
                  .transpose(1, 0, 2).reshape(128, 512))
            agg_pack[idx] = np.concatenate([w1, wh, wo], axis=1).astype(NP_BF16)
            idx += 1

    head_pack = np.concatenate(
        [full["head_w1"], full["head_wh"], full["head_wo"]], axis=1
    ).astype(NP_BF16)

    def t2(b):     # [n, 128] -> [128, n]
        return np.ascontiguousarray(b.T)

    def t4(b):     # [n, 512] -> [128, 4n] laid out as (c, i)
        n = b.shape[0]
        return b.reshape(n, 4, 128).transpose(2, 1, 0).reshape(128, 4 * n)

    hbo = np.zeros((128, 1), f32)
    hbo[0:A, 0] = full["head_bo"]
    bias_pack = np.concatenate([
        t2(full["enc_b1"]), t2(full["enc_bh"]), t2(full["enc_bo"]),
        t4(full["agg0_b1"]), t4(full["agg0_bh"]), t2(full["agg0_bo"]),
        t4(full["agg1_b1"]), t4(full["agg1_bh"]), t2(full["agg1_bo"]),
        t4(full["agg2_b1"]), t4(full["agg2_bh"]), t2(full["agg2_bo"]),
        full["head_b1"][:, None], full["head_bh"][:, None], hbo,
    ], axis=1).astype(f32)
    assert bias_pack.shape == (128, NBIAS)

    return {
        "enc_pack": enc_pack,
        "agg_pack": agg_pack,
        "head_pack": np.ascontiguousarray(head_pack),
        "bias_pack": np.ascontiguousarray(bias_pack),
    }


_NC_CACHE = None


def _get_nc():
    global _NC_CACHE
    if _NC_CACHE is None:
        _NC_CACHE = build_kernel()
    return _NC_CACHE


def run_sharded(inputs, trace=False, tmpdir=None):
    """inputs: dict of full-size np arrays. Returns (embeds, action, results)."""
    nc = _get_nc()
    full = {
        k: np.ascontiguousarray(np.asarray(v, dtype=np.float32))
        for k, v in inputs.items()
    }
    packed = pack_inputs(full)
    in_maps = []
    for i in range(N_CORES):
        st = full["state"][i * BL:(i + 1) * BL]            # [BL, S]
        state_t = np.ascontiguousarray(st.T).reshape(2, 128, BL).astype(NP_BF16)
        m = dict(packed)
        m["state_t"] = state_t
        in_maps.append(m)
    res = run_bass_kernel_spmd(
        nc, in_maps, core_ids=list(range(N_CORES)), trace=trace, tmpdir=tmpdir
    )
    embeds = np.concatenate([res.results[i]["embeds"] for i in range(N_CORES)], axis=0)
    action = np.concatenate([res.results[i]["action"] for i in range(N_CORES)], axis=0)
    return embeds, action, res


def kernel(**inputs):
    embeds, action, _ = run_sharded(inputs)
    return embeds, action


# revision 13
# speedup vs baseline: 2.1486x; 1.0100x over previous
"""Trainium2 Bass kernel for nn_Actor (gnn_message_passing tree aggregation).

Strategy:
  - Data-parallel over batch B=4096 across 8 NeuronCores (512 rows each);
    weights replicated. No collectives - pure SPMD.
  - Activations live in SBUF "transposed" (feature on partitions, batch on
    free dim). Tree concat of 4 sibling embeddings along the feature dim is
    then free: the 4 child tiles are directly the 4 K-chunks of the parent
    aggregator's first matmul.
  - Matmuls run in bf16 (fp32 PSUM accumulate). Weights are pre-cast and
    pre-packed on the host into the exact SBUF layout (one contiguous
    [128, X] bundle per tree group), halving weight DMA traffic and making
    every weight DMA a single large contiguous transfer. Biases are
    pre-transposed on the host into one [128, 384] f32 bundle.
  - Weight bundles are streamed from HBM group-by-group, triple buffered.
  - Output embeds [B,85,128] are batch-major, so each node embedding is
    PE-transposed (via identity matmul) back to batch-major before DMA out.
  - Bias+ReLU epilogues alternate between ScalarE (activation) and VectorE
    (fused tensor_scalar add+max) to balance the two engines.
"""

import numpy as np
import ml_dtypes

import concourse.bass as bass
import concourse.tile as tile
from concourse import bacc, mybir
from concourse.bass import ts
from concourse.bass_utils import run_bass_kernel_spmd
from concourse.masks import make_identity

AF = mybir.ActivationFunctionType
ALU = mybir.AluOpType
F32 = mybir.dt.float32
BF16 = mybir.dt.bfloat16
NP_BF16 = np.dtype(ml_dtypes.bfloat16)

# Problem shapes (hardcoded; must match the reference)
B, S, D, E, F, A = 4096, 256, 128, 64, 4, 32
CD = D * F                      # 512
NODES = 85                      # 64 enc + 16 + 4 + 1
N_CORES = 8
BL = B // N_CORES               # 512 rows per core
NBC = BL // 128                 # 4 batch chunks of 128

# bias bundle column offsets (host packing must match kernel reads)
EB1, EBH, EBO = 0, 64, 128
A0B1, A0BH, A0BO = 192, 256, 320
A1B1, A1BH, A1BO = 336, 352, 368
A2B1, A2BH, A2BO = 372, 376, 380
HB1, HBH, HBO = 381, 382, 383
NBIAS = 384

DRAM_SPECS = {
    "state_t": ((2, 128, BL), BF16),
    "enc_pack": ((16, 128, 2048), BF16),
    "agg_pack": ((21, 128, 4608), BF16),
    "head_pack": ((128, 288), BF16),
    "bias_pack": ((128, NBIAS), F32),
}


def build_kernel():
    nc = bacc.Bacc("TRN2", target_bir_lowering=False, debug=False)

    dram = {}
    for name, (shape, dt) in DRAM_SPECS.items():
        dram[name] = nc.dram_tensor(name, list(shape), dt, kind="ExternalInput").ap()
    embeds = nc.dram_tensor("embeds", [BL, NODES, D], F32, kind="ExternalOutput").ap()
    action = nc.dram_tensor("action", [BL, A], F32, kind="ExternalOutput").ap()

    # batch-chunked views of the outputs: [128, chunk, node, d]
    embeds_v = embeds.rearrange("(c p) n d -> p c n d", p=128)
    action_v = action.rearrange("(c p) a -> p c a", p=128)

    with tile.TileContext(nc) as tc:
        with (
            tc.tile_pool(name="const", bufs=1) as const,
            tc.tile_pool(name="wenc", bufs=4) as wenc,
            tc.tile_pool(name="wagg", bufs=4) as wagg,
            tc.tile_pool(name="encout", bufs=12) as encout,
            tc.tile_pool(name="hid", bufs=8) as hid,
            tc.tile_pool(name="xpool", bufs=12) as xpool,
            tc.tile_pool(name="a0out", bufs=16) as a0out,
            tc.tile_pool(name="a1out", bufs=5) as a1out,
            tc.tile_pool(name="smallp", bufs=1) as smallp,
            tc.tile_pool(name="natp", bufs=5) as natp,
            tc.tile_pool(name="anatp", bufs=4) as anatp,
            tc.tile_pool(name="psmm", bufs=6, space="PSUM") as psmm,
            tc.tile_pool(name="pstr", bufs=2, space="PSUM") as pstr,
        ):
            ident = const.tile([128, 128], BF16)
            make_identity(nc, ident)

            warm_ps = psmm.tile([128, BL], F32, tag="ps")
            for w in range(48):
                nc.tensor.matmul(
                    warm_ps[:, 0:128], ident, ident, start=True, stop=True
                )
            warm_out = smallp.tile([128, 8], BF16, tag="warm")
            nc.vector.tensor_copy(warm_out, warm_ps[:, 0:8])
            warm_dram = nc.dram_tensor("warm_scratch", [128, 8], BF16).ap()
            nc.sync.dma_start(warm_dram, warm_out)

            stT = const.tile([128, 2, BL], BF16, tag="stT")
            nc.sync.dma_start(stT, dram["state_t"].rearrange("s p b -> p s b"))
            bias = const.tile([128, NBIAS], F32, tag="bias")
            nc.sync.dma_start(bias, dram["bias_pack"])

            # --- helpers -------------------------------------------------
            act_counter = [0]

            def act_bias(out, ps, bias_ap, relu):
                """bias-add (+ReLU) epilogue, alternating ScalarE / VectorE."""
                i = act_counter[0]
                act_counter[0] += 1
                if i % 5 < 2:
                    if relu:
                        nc.vector.tensor_scalar(
                            out, ps, bias_ap, 0.0, ALU.add, ALU.max
                        )
                    else:
                        nc.vector.tensor_scalar_add(out, ps, bias_ap)
                else:
                    nc.scalar.activation(
                        out, ps, AF.Relu if relu else AF.Identity, bias=bias_ap
                    )

            def mm_acc(lhs_chunks, rhs_list, out_par=128):
                ps = psmm.tile([128, BL], F32, tag="ps")
                nk = len(rhs_list)
                for k in range(nk):
                    nc.tensor.matmul(
                        ps[0:out_par, :], lhs_chunks[k], rhs_list[k],
                        start=(k == 0), stop=(k == nk - 1),
                    )
                return ps

            def transpose_to_nat(src_T, dst_nat, valid=128):
                """PE-transpose [feat, b] -> batch-major [128, NBC, feat]."""
                tp = pstr.tile([128, NBC, 128], BF16, tag="tps")
                for c in range(NBC):
                    nc.tensor.transpose(
                        tp[:, c, 0:valid], src_T[0:valid, ts(c, 128)],
                        ident[0:valid, 0:valid],
                    )
                nc.vector.tensor_copy(dst_nat, tp[:, :, 0:valid])

            def fnn_3layer(x_list, aggw, boff, gi, nloc, out_pool, out_tag):
                """Aggregator FNN CD->CD->CD->D on transposed acts.
                aggw: [128, 4608] bundle; boff: (b1,bh,bo) col offsets;
                gi: node index in level; nloc: nodes in level."""
                w1 = aggw[:, 0:2048].rearrange("p (k m) -> p k m", k=4)
                wh = aggw[:, 2048:4096].rearrange("p (k m) -> p k m", k=4)
                wo = aggw[:, 4096:4608].rearrange("p (k m) -> p k m", k=4)
                ob1, obh, obo = boff
                x1 = []
                for m in range(4):
                    ps = mm_acc([w1[:, k, ts(m, 128)] for k in range(4)], x_list)
                    t = xpool.tile([128, BL], BF16, tag="x1")
                    c = ob1 + m * nloc + gi
                    act_bias(t, ps, bias[:, c:c + 1], relu=True)
                    x1.append(t)
                x2 = []
                for m in range(4):
                    ps = mm_acc([wh[:, k, ts(m, 128)] for k in range(4)], x1)
                    t = xpool.tile([128, BL], BF16, tag="x2")
                    c = obh + m * nloc + gi
                    act_bias(t, ps, bias[:, c:c + 1], relu=True)
                    x2.append(t)
                ps = mm_acc([wo[:, k, :] for k in range(4)], x2)
                out = out_pool.tile([128, BL], BF16, tag=out_tag)
                act_bias(out, ps, bias[:, obo + gi:obo + gi + 1], relu=False)
                return out

            # ================= encoders + level-0 aggregators =============
            a0_tiles = []
            a0nat = None
            for g in range(16):
                encw = wenc.tile([128, 2048], BF16, tag="encw")
                nc.sync.dma_start(encw, dram["enc_pack"][g])
                ew1 = encw[:, 0:1024].rearrange("p (e c f) -> p e c f", e=4, c=2)
                ewh = encw[:, 1024:1536].rearrange("p (e f) -> p e f", e=4)
                ewo = encw[:, 1536:2048].rearrange("p (e f) -> p e f", e=4)

                # breadth-first over the 4 encoders: keeps 4 independent
                # matmul chains in the scheduler window so PE never stalls
                # on a single epilogue.
                h1s, h2s, e_tiles = [], [], []
                for e in range(4):
                    n = 4 * g + e
                    ps = mm_acc(
                        [ew1[:, e, c, :] for c in range(2)],
                        [stT[:, c, :] for c in range(2)],
                    )
                    h1 = hid.tile([128, BL], BF16, tag="h1")
                    act_bias(h1, ps, bias[:, EB1 + n:EB1 + n + 1], relu=True)
                    h1s.append(h1)
                for e in range(4):
                    n = 4 * g + e
                    ps = mm_acc([ewh[:, e, :]], [h1s[e]])
                    h2 = hid.tile([128, BL], BF16, tag="h2")
                    act_bias(h2, ps, bias[:, EBH + n:EBH + n + 1], relu=True)
                    h2s.append(h2)
                for e in range(4):
                    n = 4 * g + e
                    ps = mm_acc([ewo[:, e, :]], [h2s[e]])
                    eT = encout.tile([128, BL], BF16, tag="eT")
                    act_bias(eT, ps, bias[:, EBO + n:EBO + n + 1], relu=False)
                    e_tiles.append(eT)

                # -- level-0 aggregator for this group (critical path) --
                aggw = wagg.tile([128, 4608], BF16, tag="aggw")
                nc.sync.dma_start(aggw, dram["agg_pack"][g])
                aT = fnn_3layer(
                    e_tiles, aggw, (A0B1, A0BH, A0BO), g, 16, a0out, "a0"
                )

                # encoder embeds output (off the critical path: emitted after
                # the aggregator so its matmuls get scheduler priority)
                enat = natp.tile([128, NBC, 4, 128], F32, tag="enat")
                for e in range(4):
                    transpose_to_nat(e_tiles[e], enat[:, :, e, :])
                nc.scalar.dma_start(embeds_v[:, :, ts(g, 4), :], enat)
                a0_tiles.append(aT)
                if g % 4 == 0:
                    a0nat = anatp.tile([128, NBC, 4, 128], F32, tag="anat")
                transpose_to_nat(aT, a0nat[:, :, g % 4, :])
                if g % 4 == 3:
                    nc.scalar.dma_start(
                        embeds_v[:, :, ts(16 + g // 4, 4), :], a0nat
                    )

            # ================= level-1 aggregators ========================
            a1_tiles = []
            a1nat = anatp.tile([128, NBC, 4, 128], F32, tag="anat")
            for j in range(4):
                aggw = wagg.tile([128, 4608], BF16, tag="aggw")
                nc.sync.dma_start(aggw, dram["agg_pack"][16 + j])
                aT = fnn_3layer(
                    a0_tiles[4 * j:4 * j + 4], aggw, (A1B1, A1BH, A1BO),
                    j, 4, a1out, "a1",
                )
                a1_tiles.append(aT)
                transpose_to_nat(aT, a1nat[:, :, j, :])
            nc.scalar.dma_start(embeds_v[:, :, ts(20, 4), :], a1nat)

            # ================= level-2 (root) =============================
            aggw = wagg.tile([128, 4608], BF16, tag="aggw")
            nc.sync.dma_start(aggw, dram["agg_pack"][20])
            rootT = fnn_3layer(
                a1_tiles, aggw, (A2B1, A2BH, A2BO), 0, 1, smallp, "root"
            )
            rnat = smallp.tile([128, NBC, 1, 128], F32, tag="rnat")
            transpose_to_nat(rootT, rnat[:, :, 0, :])
            nc.scalar.dma_start(embeds_v[:, :, 84:85, :], rnat)

            # ================= head =======================================
            headw = smallp.tile([128, 288], BF16, tag="headw")
            nc.sync.dma_start(headw, dram["head_pack"])
            hw1, hwh, hwo = headw[:, 0:128], headw[:, 128:256], headw[:, 256:288]

            ps = mm_acc([hw1], [rootT])
            hh1 = hid.tile([128, BL], BF16, tag="h1")
            act_bias(hh1, ps, bias[:, HB1:HB1 + 1], relu=True)
            ps = mm_acc([hwh], [hh1])
            hh2 = hid.tile([128, BL], BF16, tag="h2")
            act_bias(hh2, ps, bias[:, HBH:HBH + 1], relu=True)
            ps = mm_acc([hwo], [hh2], out_par=A)
            actT = smallp.tile([A, BL], BF16, tag="actT")
            nc.scalar.activation(
                actT, ps[0:A, :], AF.Tanh, bias=bias[0:A, HBO:HBO + 1]
            )

            anat = smallp.tile([128, NBC, A], F32, tag="act_nat")
            tp = pstr.tile([128, NBC, 128], BF16, tag="tps")
            for c in range(NBC):
                nc.tensor.transpose(
                    tp[:, c, 0:A], actT[:, ts(c, 128)], ident[0:A, 0:A]
                )
            nc.vector.tensor_copy(anat, tp[:, :, 0:A])
            nc.scalar.dma_start(action_v, anat)

    nc.compile()
    return nc


def pack_inputs(full):
    """Host-side: cast+pack weights into SBUF-layout bundles (bf16) and
    biases into one transposed f32 bundle."""
    f32 = np.float32
    enc_pack = np.empty((16, 128, 2048), NP_BF16)
    for g in range(16):
        sl = slice(4 * g, 4 * g + 4)
        ew1 = (full["enc_w1"][sl].reshape(4, 2, 128, 128)
               .transpose(2, 0, 1, 3).reshape(128, 1024))
        ewh = full["enc_wh"][sl].transpose(1, 0, 2).reshape(128, 512)
        ewo = full["enc_wo"][sl].transpose(1, 0, 2).reshape(128, 512)
        enc_pack[g] = np.concatenate([ew1, ewh, ewo], axis=1).astype(NP_BF16)

    agg_pack = np.empty((21, 128, 4608), NP_BF16)
    idx = 0
    for lvl in ("agg0", "agg1", "agg2"):
        for gi in range(full[f"{lvl}_w1"].shape[0]):
            w1 = (full[f"{lvl}_w1"][gi].reshape(4, 128, 512)
                  .transpose(1, 0, 2).reshape(128, 2048))
            wh = (full[f"{lvl}_wh"][gi].reshape(4, 128, 512)
                  .transpose(1, 0, 2).reshape(128, 2048))
            wo = (full[f"{lvl}_wo"][gi].reshape(4, 128, 128)
                  .transpose(1, 0, 2).reshape(128, 512))
            agg_pack[idx] = np.concatenate([w1, wh, wo], axis=1).astype(NP_BF16)
            idx += 1

    head_pack = np.concatenate(
        [full["head_w1"], full["head_wh"], full["head_wo"]], axis=1
    ).astype(NP_BF16)

    def t2(b):     # [n, 128] -> [128, n]
        return np.ascontiguousarray(b.T)

    def t4(b):     # [n, 512] -> [128, 4n] laid out as (c, i)
        n = b.shape[0]
        return b.reshape(n, 4, 128).transpose(2, 1, 0).reshape(128, 4 * n)

    hbo = np.zeros((128, 1), f32)
    hbo[0:A, 0] = full["head_bo"]
    bias_pack = np.concatenate([
        t2(full["enc_b1"]), t2(full["enc_bh"]), t2(full["enc_bo"]),
        t4(full["agg0_b1"]), t4(full["agg0_bh"]), t2(full["agg0_bo"]),
        t4(full["agg1_b1"]), t4(full["agg1_bh"]), t2(full["agg1_bo"]),
        t4(full["agg2_b1"]), t4(full["agg2_bh"]), t2(full["agg2_bo"]),
        full["head_b1"][:, None], full["head_bh"][:, None], hbo,
    ], axis=1).astype(f32)
    assert bias_pack.shape == (128, NBIAS)

    return {
        "enc_pack": enc_pack,
        "agg_pack": agg_pack,
        "head_pack": np.ascontiguousarray(head_pack),
        "bias_pack": np.ascontiguousarray(bias_pack),
    }


_NC_CACHE = None


def _get_nc():
    global _NC_CACHE
    if _NC_CACHE is None:
        _NC_CACHE = build_kernel()
    return _NC_CACHE


def run_sharded(inputs, trace=False, tmpdir=None):
    """inputs: dict of full-size np arrays. Returns (embeds, action, results)."""
    nc = _get_nc()
    full = {
        k: np.ascontiguousarray(np.asarray(v, dtype=np.float32))
        for k, v in inputs.items()
    }
    packed = pack_inputs(full)
    in_maps = []
    for i in range(N_CORES):
        st = full["state"][i * BL:(i + 1) * BL]            # [BL, S]
        state_t = np.ascontiguousarray(st.T).reshape(2, 128, BL).astype(NP_BF16)
        m = dict(packed)
        m["state_t"] = state_t
        in_maps.append(m)
    res = run_bass_kernel_spmd(
        nc, in_maps, core_ids=list(range(N_CORES)), trace=trace, tmpdir=tmpdir
    )
    embeds = np.concatenate([res.results[i]["embeds"] for i in range(N_CORES)], axis=0)
    action = np.concatenate([res.results[i]["action"] for i in range(N_CORES)], axis=0)
    return embeds, action, res


def kernel(**inputs):
    embeds, action, _ = run_sharded(inputs)
    return embeds, action


# revision 14
# speedup vs baseline: 2.4451x; 1.1380x over previous
"""Trainium2 Bass kernel for nn_Actor (gnn_message_passing tree aggregation).

Strategy:
  - Data-parallel over batch B=4096 across 8 NeuronCores (512 rows each);
    weights replicated. No collectives - pure SPMD.
  - Activations live in SBUF "transposed" (feature on partitions, batch on
    free dim). Tree concat of 4 sibling embeddings along the feature dim is
    then free: the 4 child tiles are directly the 4 K-chunks of the parent
    aggregator's first matmul.
  - Matmuls run in bf16 (fp32 PSUM accumulate). Weights are pre-cast and
    pre-packed on the host into the exact SBUF layout (one contiguous
    [128, X] bundle per tree group), halving weight DMA traffic and making
    every weight DMA a single large contiguous transfer. Biases are
    pre-transposed on the host into one [128, 384] f32 bundle.
  - Weight bundles are streamed from HBM group-by-group, triple buffered.
  - Output embeds [B,85,128] are batch-major, so each node embedding is
    PE-transposed (via identity matmul) back to batch-major before DMA out.
  - Bias+ReLU epilogues alternate between ScalarE (activation) and VectorE
    (fused tensor_scalar add+max) to balance the two engines.
"""

import numpy as np
import ml_dtypes

import concourse.bass as bass
import concourse.tile as tile
from concourse import bacc, mybir
from concourse.bass import ts
from concourse.bass_utils import run_bass_kernel_spmd
from concourse.masks import make_identity

AF = mybir.ActivationFunctionType
ALU = mybir.AluOpType
F32 = mybir.dt.float32
BF16 = mybir.dt.bfloat16
NP_BF16 = np.dtype(ml_dtypes.bfloat16)

# Problem shapes (hardcoded; must match the reference)
B, S, D, E, F, A = 4096, 256, 128, 64, 4, 32
CD = D * F                      # 512
NODES = 85                      # 64 enc + 16 + 4 + 1
N_CORES = 8
BL = B // N_CORES               # 512 rows per core
NBC = BL // 128                 # 4 batch chunks of 128

# bias bundle column offsets (host packing must match kernel reads)
EB1, EBH, EBO = 0, 64, 128
A0B1, A0BH, A0BO = 192, 256, 320
A1B1, A1BH, A1BO = 336, 352, 368
A2B1, A2BH, A2BO = 372, 376, 380
HB1, HBH, HBO = 381, 382, 383
NBIAS = 384

DRAM_SPECS = {
    "state_t": ((2, 128, BL), BF16),
    "enc_pack": ((16, 128, 2048), BF16),
    "agg_pack": ((21, 128, 4608), BF16),
    "head_pack": ((128, 288), BF16),
    "bias_pack": ((128, NBIAS), F32),
}


def build_kernel():
    nc = bacc.Bacc("TRN2", target_bir_lowering=False, debug=False)

    dram = {}
    for name, (shape, dt) in DRAM_SPECS.items():
        dram[name] = nc.dram_tensor(name, list(shape), dt, kind="ExternalInput").ap()
    embeds = nc.dram_tensor("embeds", [BL, NODES, D], F32, kind="ExternalOutput").ap()
    action = nc.dram_tensor("action", [BL, A], F32, kind="ExternalOutput").ap()

    # batch-chunked views of the outputs: [128, chunk, node, d]
    embeds_v = embeds.rearrange("(c p) n d -> p c n d", p=128)
    action_v = action.rearrange("(c p) a -> p c a", p=128)

    with tile.TileContext(nc) as tc:
        with (
            tc.tile_pool(name="const", bufs=1) as const,
            tc.tile_pool(name="wenc", bufs=4) as wenc,
            tc.tile_pool(name="wagg", bufs=4) as wagg,
            tc.tile_pool(name="encout", bufs=12) as encout,
            tc.tile_pool(name="hid", bufs=8) as hid,
            tc.tile_pool(name="xpool", bufs=12) as xpool,
            tc.tile_pool(name="a0out", bufs=16) as a0out,
            tc.tile_pool(name="a1out", bufs=5) as a1out,
            tc.tile_pool(name="smallp", bufs=1) as smallp,
            tc.tile_pool(name="natp", bufs=5) as natp,
            tc.tile_pool(name="anatp", bufs=4) as anatp,
            tc.tile_pool(name="psmm", bufs=6, space="PSUM") as psmm,
            tc.tile_pool(name="pstr", bufs=2, space="PSUM") as pstr,
        ):
            ident = const.tile([128, 128], BF16)
            make_identity(nc, ident)

            warm_ps = psmm.tile([128, BL], F32, tag="ps")
            for w in range(48):
                nc.tensor.matmul(
                    warm_ps[:, 0:128], ident, ident, start=True, stop=True
                )
            warm_out = smallp.tile([128, 8], BF16, tag="warm")
            nc.vector.tensor_copy(warm_out, warm_ps[:, 0:8])
            warm_dram = nc.dram_tensor("warm_scratch", [128, 8], BF16).ap()
            nc.sync.dma_start(warm_dram, warm_out)

            stT = const.tile([128, 2, BL], BF16, tag="stT")
            nc.sync.dma_start(stT, dram["state_t"].rearrange("s p b -> p s b"))
            bias = const.tile([128, NBIAS], F32, tag="bias")
            nc.sync.dma_start(bias, dram["bias_pack"])

            # --- helpers -------------------------------------------------
            act_counter = [0]

            def act_bias(out, ps, bias_ap, relu):
                """bias-add (+ReLU) epilogue, alternating ScalarE / VectorE."""
                i = act_counter[0]
                act_counter[0] += 1
                if i % 5 < 2:
                    if relu:
                        nc.vector.tensor_scalar(
                            out, ps, bias_ap, 0.0, ALU.add, ALU.max
                        )
                    else:
                        nc.vector.tensor_scalar_add(out, ps, bias_ap)
                else:
                    nc.scalar.activation(
                        out, ps, AF.Relu if relu else AF.Identity, bias=bias_ap
                    )

            def mm_acc(lhs_chunks, rhs_list, out_par=128):
                ps = psmm.tile([128, BL], F32, tag="ps")
                nk = len(rhs_list)
                for k in range(nk):
                    nc.tensor.matmul(
                        ps[0:out_par, :], lhs_chunks[k], rhs_list[k],
                        start=(k == 0), stop=(k == nk - 1),
                    )
                return ps

            def transpose_to_nat(src_T, dst_nat, valid=128):
                """PE-transpose [feat, b] -> batch-major [128, NBC, feat]."""
                tp = pstr.tile([128, NBC, 128], BF16, tag="tps")
                for c in range(NBC):
                    nc.tensor.transpose(
                        tp[:, c, 0:valid], src_T[0:valid, ts(c, 128)],
                        ident[0:valid, 0:valid],
                    )
                nc.vector.tensor_copy(dst_nat, tp[:, :, 0:valid])

            def fnn_3layer(x_list, aggw, boff, gi, nloc, out_pool, out_tag):
                """Aggregator FNN CD->CD->CD->D on transposed acts.
                aggw: [128, 4608] bundle; boff: (b1,bh,bo) col offsets;
                gi: node index in level; nloc: nodes in level."""
                w1 = aggw[:, 0:2048].rearrange("p (k m) -> p k m", k=4)
                wh = aggw[:, 2048:4096].rearrange("p (k m) -> p k m", k=4)
                wo = aggw[:, 4096:4608].rearrange("p (k m) -> p k m", k=4)
                ob1, obh, obo = boff
                x1 = []
                for m in range(4):
                    ps = mm_acc([w1[:, k, ts(m, 128)] for k in range(4)], x_list)
                    t = xpool.tile([128, BL], BF16, tag="x1")
                    c = ob1 + m * nloc + gi
                    act_bias(t, ps, bias[:, c:c + 1], relu=True)
                    x1.append(t)
                x2 = []
                for m in range(4):
                    ps = mm_acc([wh[:, k, ts(m, 128)] for k in range(4)], x1)
                    t = xpool.tile([128, BL], BF16, tag="x2")
                    c = obh + m * nloc + gi
                    act_bias(t, ps, bias[:, c:c + 1], relu=True)
                    x2.append(t)
                ps = mm_acc([wo[:, k, :] for k in range(4)], x2)
                out = out_pool.tile([128, BL], BF16, tag=out_tag)
                act_bias(out, ps, bias[:, obo + gi:obo + gi + 1], relu=False)
                return out

            # ================= encoders + level-0 aggregators =============
            a0_tiles = []
            a0nat = None
            pending = []
            for g in range(16):
                encw = wenc.tile([128, 2048], BF16, tag="encw")
                nc.sync.dma_start(encw, dram["enc_pack"][g])
                ew1 = encw[:, 0:1024].rearrange("p (e c f) -> p e c f", e=4, c=2)
                ewh = encw[:, 1024:1536].rearrange("p (e f) -> p e f", e=4)
                ewo = encw[:, 1536:2048].rearrange("p (e f) -> p e f", e=4)

                # breadth-first over the 4 encoders: keeps 4 independent
                # matmul chains in the scheduler window so PE never stalls
                # on a single epilogue.
                h1s, h2s, e_tiles = [], [], []
                for e in range(4):
                    n = 4 * g + e
                    ps = mm_acc(
                        [ew1[:, e, c, :] for c in range(2)],
                        [stT[:, c, :] for c in range(2)],
                    )
                    h1 = hid.tile([128, BL], BF16, tag="h1")
                    act_bias(h1, ps, bias[:, EB1 + n:EB1 + n + 1], relu=True)
                    h1s.append(h1)
                for e in range(4):
                    n = 4 * g + e
                    ps = mm_acc([ewh[:, e, :]], [h1s[e]])
                    h2 = hid.tile([128, BL], BF16, tag="h2")
                    act_bias(h2, ps, bias[:, EBH + n:EBH + n + 1], relu=True)
                    h2s.append(h2)
                for e in range(4):
                    n = 4 * g + e
                    ps = mm_acc([ewo[:, e, :]], [h2s[e]])
                    eT = encout.tile([128, BL], BF16, tag="eT")
                    act_bias(eT, ps, bias[:, EBO + n:EBO + n + 1], relu=False)
                    e_tiles.append(eT)

                # -- level-0 aggregator for this group (critical path) --
                aggw = wagg.tile([128, 4608], BF16, tag="aggw")
                nc.sync.dma_start(aggw, dram["agg_pack"][g])
                aT = fnn_3layer(
                    e_tiles, aggw, (A0B1, A0BH, A0BO), g, 16, a0out, "a0"
                )

                # embeds-output work for this group is deferred one full
                # group so it never outranks critical epilogues in the
                # ScalarE/VectorE FIFOs.
                for fn in pending:
                    fn()
                pending = []

                def emit_enc_out(e_tiles=e_tiles, g=g):
                    enat = natp.tile([128, NBC, 4, 128], BF16, tag="enat")
                    for e in range(4):
                        transpose_to_nat(e_tiles[e], enat[:, :, e, :])
                    nc.gpsimd.dma_start(embeds_v[:, :, ts(g, 4), :], enat)

                pending.append(emit_enc_out)
                a0_tiles.append(aT)

                def emit_a0_out(aT=aT, g=g):
                    nonlocal a0nat
                    if g % 4 == 0:
                        a0nat = anatp.tile([128, NBC, 4, 128], BF16, tag="anat")
                    transpose_to_nat(aT, a0nat[:, :, g % 4, :])
                    if g % 4 == 3:
                        nc.gpsimd.dma_start(
                            embeds_v[:, :, ts(16 + g // 4, 4), :], a0nat
                        )

                pending.append(emit_a0_out)

            for fn in pending:
                fn()
            pending = []

            # ================= level-1 aggregators ========================
            a1_tiles = []
            a1nat = anatp.tile([128, NBC, 4, 128], BF16, tag="anat")
            for j in range(4):
                aggw = wagg.tile([128, 4608], BF16, tag="aggw")
                nc.sync.dma_start(aggw, dram["agg_pack"][16 + j])
                aT = fnn_3layer(
                    a0_tiles[4 * j:4 * j + 4], aggw, (A1B1, A1BH, A1BO),
                    j, 4, a1out, "a1",
                )
                a1_tiles.append(aT)
                transpose_to_nat(aT, a1nat[:, :, j, :])
            nc.gpsimd.dma_start(embeds_v[:, :, ts(20, 4), :], a1nat)

            # ================= level-2 (root) =============================
            aggw = wagg.tile([128, 4608], BF16, tag="aggw")
            nc.sync.dma_start(aggw, dram["agg_pack"][20])
            rootT = fnn_3layer(
                a1_tiles, aggw, (A2B1, A2BH, A2BO), 0, 1, smallp, "root"
            )
            rnat = smallp.tile([128, NBC, 1, 128], BF16, tag="rnat")
            transpose_to_nat(rootT, rnat[:, :, 0, :])
            nc.gpsimd.dma_start(embeds_v[:, :, 84:85, :], rnat)

            # ================= head =======================================
            headw = smallp.tile([128, 288], BF16, tag="headw")
            nc.sync.dma_start(headw, dram["head_pack"])
            hw1, hwh, hwo = headw[:, 0:128], headw[:, 128:256], headw[:, 256:288]

            ps = mm_acc([hw1], [rootT])
            hh1 = hid.tile([128, BL], BF16, tag="h1")
            act_bias(hh1, ps, bias[:, HB1:HB1 + 1], relu=True)
            ps = mm_acc([hwh], [hh1])
            hh2 = hid.tile([128, BL], BF16, tag="h2")
            act_bias(hh2, ps, bias[:, HBH:HBH + 1], relu=True)
            ps = mm_acc([hwo], [hh2], out_par=A)
            actT = smallp.tile([A, BL], BF16, tag="actT")
            nc.scalar.activation(
                actT, ps[0:A, :], AF.Tanh, bias=bias[0:A, HBO:HBO + 1]
            )

            anat = smallp.tile([128, NBC, A], BF16, tag="act_nat")
            tp = pstr.tile([128, NBC, 128], BF16, tag="tps")
            for c in range(NBC):
                nc.tensor.transpose(
                    tp[:, c, 0:A], actT[:, ts(c, 128)], ident[0:A, 0:A]
                )
            nc.vector.tensor_copy(anat, tp[:, :, 0:A])
            nc.gpsimd.dma_start(action_v, anat)

    nc.compile()
    return nc


def pack_inputs(full):
    """Host-side: cast+pack weights into SBUF-layout bundles (bf16) and
    biases into one transposed f32 bundle."""
    f32 = np.float32
    enc_pack = np.empty((16, 128, 2048), NP_BF16)
    for g in range(16):
        sl = slice(4 * g, 4 * g + 4)
        ew1 = (full["enc_w1"][sl].reshape(4, 2, 128, 128)
               .transpose(2, 0, 1, 3).reshape(128, 1024))
        ewh = full["enc_wh"][sl].transpose(1, 0, 2).reshape(128, 512)
        ewo = full["enc_wo"][sl].transpose(1, 0, 2).reshape(128, 512)
        enc_pack[g] = np.concatenate([ew1, ewh, ewo], axis=1).astype(NP_BF16)

    agg_pack = np.empty((21, 128, 4608), NP_BF16)
    idx = 0
    for lvl in ("agg0", "agg1", "agg2"):
        for gi in range(full[f"{lvl}_w1"].shape[0]):
            w1 = (full[f"{lvl}_w1"][gi].reshape(4, 128, 512)
                  .transpose(1, 0, 2).reshape(128, 2048))
            wh = (full[f"{lvl}_wh"][gi].reshape(4, 128, 512)
                  .transpose(1, 0, 2).reshape(128, 2048))
            wo = (full[f"{lvl}_wo"][gi].reshape(4, 128, 128)
                  .transpose(1, 0, 2).reshape(128, 512))
            agg_pack[idx] = np.concatenate([w1, wh, wo], axis=1).astype(NP_BF16)
            idx += 1

    head_pack = np.concatenate(
        [full["head_w1"], full["head_wh"], full["head_wo"]], axis=1
    ).astype(NP_BF16)

    def t2(b):     # [n, 128] -> [128, n]
        return np.ascontiguousarray(b.T)

    def t4(b):     # [n, 512] -> [128, 4n] laid out as (c, i)
        n = b.shape[0]
        return b.reshape(n, 4, 128).transpose(2, 1, 0).reshape(128, 4 * n)

    hbo = np.zeros((128, 1), f32)
    hbo[0:A, 0] = full["head_bo"]
    bias_pack = np.concatenate([
        t2(full["enc_b1"]), t2(full["enc_bh"]), t2(full["enc_bo"]),
        t4(full["agg0_b1"]), t4(full["agg0_bh"]), t2(full["agg0_bo"]),
        t4(full["agg1_b1"]), t4(full["agg1_bh"]), t2(full["agg1_bo"]),
        t4(full["agg2_b1"]), t4(full["agg2_bh"]), t2(full["agg2_bo"]),
        full["head_b1"][:, None], full["head_bh"][:, None], hbo,
    ], axis=1).astype(f32)
    assert bias_pack.shape == (128, NBIAS)

    return {
        "enc_pack": enc_pack,
        "agg_pack": agg_pack,
        "head_pack": np.ascontiguousarray(head_pack),
        "bias_pack": np.ascontiguousarray(bias_pack),
    }


_NC_CACHE = None


def _get_nc():
    global _NC_CACHE
    if _NC_CACHE is None:
        _NC_CACHE = build_kernel()
    return _NC_CACHE


def run_sharded(inputs, trace=False, tmpdir=None):
    """inputs: dict of full-size np arrays. Returns (embeds, action, results)."""
    nc = _get_nc()
    full = {
        k: np.ascontiguousarray(np.asarray(v, dtype=np.float32))
        for k, v in inputs.items()
    }
    packed = pack_inputs(full)
    in_maps = []
    for i in range(N_CORES):
        st = full["state"][i * BL:(i + 1) * BL]            # [BL, S]
        state_t = np.ascontiguousarray(st.T).reshape(2, 128, BL).astype(NP_BF16)
        m = dict(packed)
        m["state_t"] = state_t
        in_maps.append(m)
    res = run_bass_kernel_spmd(
        nc, in_maps, core_ids=list(range(N_CORES)), trace=trace, tmpdir=tmpdir
    )
    embeds = np.concatenate([res.results[i]["embeds"] for i in range(N_CORES)], axis=0)
    action = np.concatenate([res.results[i]["action"] for i in range(N_CORES)], axis=0)
    return embeds, action, res


def kernel(**inputs):
    embeds, action, _ = run_sharded(inputs)
    return embeds, action
